# revision 2
# baseline (speedup 1.0000x reference)
"""Causal self-attention TRN2 Bass kernel.

Problem: B=4, T=2048, C=1024, H=16 heads, D=64 (fp32 I/O).

Sharding (8 cores): core i handles batch b = i//2 and heads
8*(i%2) .. 8*(i%2)+8  (8 heads, 512 local features).  Each core:
  qkv_local = x[b] @ W_attn[:, cols]                [2048, 512] x3
  attention over its 8 heads (causal, T=2048)
  partial_out = y_local @ W_proj[rows, :]           [2048, 1024]
Host: out[b] = partial(core 2b) + partial(core 2b+1) + bias terms.

Implementation notes (driven by measured infra costs — DMA descriptor
count and per-op counts dominate the per-call marginal on this target,
matmul streams are near cost-model speed):
  - bf16 inputs/weights/K/V/att/y/outputs; fp32 PSUM accumulation.
    Measured rel err ~3.6e-3 (tolerance 2e-2).
  - 3 DMA descriptors per iteration (2 x-loads + 1 out-store) + 5 for
    weights: x resident in SBUF [128, 8cc, 2048], out staged in SBUF
    [128, 16tb, 1024] and stored once; qkv weights ship as one tensor.
  - Merged ops: S for both heads of a pair in one 2-bank PSUM tile ->
    one exp per (j, kb); one reciprocal/partition_broadcast per j; one
    v-copy per 128-token block.
  - k-bias dropped: (q+bq)@(k+bk) differs from (q+bq)@k only by a
    per-query constant, which softmax ignores.  v/proj bias folded on
    the host into a constant output row (attention rows sum to 1).
  - QKV emission for chunk tcx+1 is woven into the attention loop of
    chunk tcx to keep the PE busy while ACT works through the exps.
"""
import numpy as np
from contextlib import ExitStack

import jax
import ml_dtypes
import concourse.bass as bass
import concourse.tile as tile
from concourse import bacc, mybir

jax.config.update("jax_compilation_cache_dir", "/tmp/jaxcache")
jax.config.update("jax_persistent_cache_min_entry_size_bytes", -1)
jax.config.update("jax_persistent_cache_min_compile_time_secs", 0.0)

B, T, C, H, D = 4, 2048, 1024, 16, 64
NCORES = 8
HPC = 8            # heads per core
FL = HPC * D       # 512 local features per core
NTC = 4            # 512-token chunks per core
NTB = 16           # 128-token blocks per core
F32 = mybir.dt.float32
BF16 = mybir.dt.bfloat16
AF = mybir.ActivationFunctionType
BF = ml_dtypes.bfloat16

_CACHED_NC = None
_CACHED_RUNNER = None


def _build(reps=1, weave=True):
    nc = bacc.Bacc("TRN2", target_bir_lowering=False, debug=False,
                   num_devices=NCORES)

    xt = nc.dram_tensor("xt", [C, T], BF16, kind="ExternalInput").ap()
    wa = nc.dram_tensor("wa", [C, 3 * FL], BF16, kind="ExternalInput").ap()
    wp = nc.dram_tensor("wp", [FL, C], BF16, kind="ExternalInput").ap()
    bq = nc.dram_tensor("bq", [128, 4], F32, kind="ExternalInput").ap()
    tri = nc.dram_tensor("tri", [128, 128], F32, kind="ExternalInput").ap()
    out = nc.dram_tensor("out", [T, C], BF16, kind="ExternalOutput").ap()

    xt_r = xt.rearrange("(cc p) t -> p cc t", p=128)
    out_r = out.rearrange("(tb p) c -> p tb c", p=128)

    with tile.TileContext(nc) as tc, ExitStack() as ctx:
        ctx.enter_context(nc.allow_low_precision(reason="bf16 matmuls"))
        singles = ctx.enter_context(tc.tile_pool(name="singles", bufs=1))
        qT_pool = ctx.enter_context(tc.tile_pool(name="qT", bufs=8))
        attT_pool = ctx.enter_context(tc.tile_pool(name="attT", bufs=4))
        yT_pool = ctx.enter_context(tc.tile_pool(name="yT", bufs=2))
        rc_pool = ctx.enter_context(tc.tile_pool(name="rc", bufs=2))
        bcs_pool = ctx.enter_context(tc.tile_pool(name="bcs", bufs=2))
        ps_a = ctx.enter_context(tc.tile_pool(name="ps_a", bufs=2, space="PSUM"))
        ps_s = ctx.enter_context(tc.tile_pool(name="ps_s", bufs=2, space="PSUM"))
        ps_y = ctx.enter_context(tc.tile_pool(name="ps_y", bufs=1, space="PSUM"))

        wa_sb = singles.tile([128, 8, 3 * FL], BF16)
        wp_sb = singles.tile([128, 4, C], BF16)
        nc.scalar.dma_start(out=wa_sb,
                            in_=wa.rearrange("(cc p) f -> p cc f", p=128))
        nc.scalar.dma_start(out=wp_sb,
                            in_=wp.rearrange("(j p) o -> p j o", p=128))
        wq_sb = wa_sb[:, :, 0:FL]
        wk_sb = wa_sb[:, :, FL:2 * FL]
        wv_sb = wa_sb[:, :, 2 * FL:3 * FL]
        bq_sb = singles.tile([128, 4], F32)
        tri_sb = singles.tile([128, 2, 128], F32)
        nc.scalar.dma_start(out=bq_sb, in_=bq)
        nc.scalar.dma_start(out=tri_sb[:, 0], in_=tri)
        nc.vector.tensor_copy(tri_sb[:, 1], tri_sb[:, 0])

        # kT: [128 (pair-feature), j (head pair), t]
        kT_sb = singles.tile([128, 4, T], BF16)
        # v: [128 (t%128), tb, head, 66]; cols 64,65 stay 1.0 -> denominator
        v_sb = singles.tile([128, NTB, HPC, 66], BF16)
        nc.vector.memset(v_sb, 1.0)
        # x resident: [128 (c%128), cc, t]
        x_sb = singles.tile([128, 8, T], BF16)
        # out staging: [128 (t%128), tb, c]
        o_sb = singles.tile([128, NTB, C], BF16)

        for rep in range(reps):
            if rep > 0:
                tc.strict_bb_all_engine_barrier()

            for half in range(2):
                nc.sync.dma_start(
                    out=x_sb[:, :, half * 1024:(half + 1) * 1024],
                    in_=xt_r[:, :, half * 1024:(half + 1) * 1024])

            qts_all = {}

            def a_units(tcx, rep=rep, qts_all=qts_all):
                """Yield thunks; each emits one QKV work unit for chunk tcx."""
                qts = qts_all[tcx] = [
                    qT_pool.tile([128, 512], BF16, tag="qT",
                                 name=f"qt{rep}_{tcx}_{j}")
                    for j in range(4)
                ]
                t0 = tcx * 512
                xs = x_sb[:, :, t0:t0 + 512]

                for j in range(4):
                    def uq(j=j, xs=xs, qts=qts):
                        p = ps_a.tile([128, 512], F32, tag="a", name="pq")
                        for cc in range(8):
                            nc.tensor.matmul(
                                p, wq_sb[:, cc, j * 128:(j + 1) * 128],
                                xs[:, cc], start=(cc == 0), stop=(cc == 7))
                        nc.vector.tensor_scalar_add(qts[j], p,
                                                    bq_sb[:, j:j + 1])
                    yield uq

                    def uk(j=j, t0=t0, xs=xs):
                        p = ps_a.tile([128, 512], F32, tag="a", name="pk")
                        for cc in range(8):
                            nc.tensor.matmul(
                                p, wk_sb[:, cc, j * 128:(j + 1) * 128],
                                xs[:, cc], start=(cc == 0), stop=(cc == 7))
                        nc.vector.tensor_copy(kT_sb[:, j, t0:t0 + 512], p)
                    yield uk

                for tb_rel in range(4):
                    def uv(tb_rel=tb_rel, tcx=tcx, xs=xs):
                        tb = tcx * 4 + tb_rel
                        pv = ps_a.tile([128, 512], F32, tag="a", name="pv")
                        for cc in range(8):
                            nc.tensor.matmul(
                                pv, xs[:, cc, tb_rel * 128:(tb_rel + 1) * 128],
                                wv_sb[:, cc, :], start=(cc == 0), stop=(cc == 7))
                        nc.vector.tensor_copy(
                            v_sb[:, tb, :, 0:64],
                            pv.rearrange("p (i d) -> p i d", i=8))
                    yield uv

            # Prologue: A(0) fully.
            for u in a_units(0):
                u()

            for tcx in range(NTC):
                qts = qts_all[tcx]
                nxt = list(a_units(tcx + 1)) if (weave and tcx + 1 < NTC) else []
                nkb = 4 * tcx + 4
                n_slots = 4 * nkb
                emitted = 0
                slot = 0

                yt = yT_pool.tile([128, 4, 512], BF16, tag="yT",
                                  name=f"yt{rep}_{tcx}")
                for j in range(4):
                    Y = ps_y.tile([66, 2, 512], F32, tag="y",
                                  name=f"Y{rep}_{tcx}_{j}")
                    for kb in range(nkb):
                        want = ((slot + 1) * len(nxt)) // n_slots
                        while emitted < want:
                            nxt[emitted]()
                            emitted += 1
                        slot += 1

                        jj = kb - 4 * tcx
                        c0 = max(jj, 0) * 128
                        S = ps_s.tile([128, 2, 512], F32, tag="s",
                                      name=f"S{rep}_{tcx}_{j}_{kb}")
                        for h in range(2):
                            nc.tensor.matmul(
                                S[:, h, c0:512],
                                kT_sb[h * 64:(h + 1) * 64, j,
                                      kb * 128:(kb + 1) * 128],
                                qts[j][h * 64:(h + 1) * 64, c0:512],
                                start=True, stop=True,
                                tile_position=(h * 64, 0))
                        att = attT_pool.tile([128, 2, 512], BF16, tag="attT",
                                             name=f"attT{rep}_{tcx}_{j}_{kb}")
                        if jj >= 0:
                            nc.vector.tensor_add(
                                S[:, :, jj * 128:(jj + 1) * 128],
                                S[:, :, jj * 128:(jj + 1) * 128], tri_sb)
                        nc.scalar.activation(att[:, :, c0:512],
                                             S[:, :, c0:512], AF.Exp)
                        for h in range(2):
                            nc.tensor.matmul(
                                Y[:, h, c0:512],
                                v_sb[:, kb, 2 * j + h, :],
                                att[:, h, c0:512],
                                start=(kb == 0), stop=(kb == nkb - 1))
                    rc = rc_pool.tile([1, 2, 512], F32, tag="rc",
                                      name=f"rc{rep}_{tcx}_{j}")
                    nc.vector.reciprocal(rc, Y[64:65, :, :])
                    bcs = bcs_pool.tile([64, 2, 512], F32, tag="bcs",
                                        name=f"bcs{rep}_{tcx}_{j}")
                    nc.gpsimd.partition_broadcast(bcs, rc)
                    for h in range(2):
                        nc.vector.tensor_mul(yt[h * 64:(h + 1) * 64, j, :],
                                             Y[0:64, h, :], bcs[:, h, :])
                while emitted < len(nxt):
                    nxt[emitted]()
                    emitted += 1

                # ---------- projection for chunk tcx ----------
                for tb_rel in range(4):
                    for oc in range(2):
                        pp = ps_a.tile([128, 512], F32, tag="a", name="pp")
                        for j in range(4):
                            nc.tensor.matmul(
                                pp, yt[:, j, tb_rel * 128:(tb_rel + 1) * 128],
                                wp_sb[:, j, oc * 512:(oc + 1) * 512],
                                start=(j == 0), stop=(j == 3))
                        nc.vector.tensor_copy(
                            o_sb[:, tcx * 4 + tb_rel,
                                 oc * 512:(oc + 1) * 512], pp)

            nc.scalar.dma_start(out=out_r, in_=o_sb)

    nc.compile()
    return nc


def _get_nc():
    global _CACHED_NC
    if _CACHED_NC is None:
        _CACHED_NC = _build()
    return _CACHED_NC


def _get_runner():
    """Compile-once jitted shard_map runner (mirrors
    bass2jax.run_bass_via_pjrt, but cached so repeat kernel() calls skip
    re-tracing)."""
    global _CACHED_RUNNER
    if _CACHED_RUNNER is not None:
        return _CACHED_RUNNER

    from jax.sharding import Mesh, PartitionSpec
    from jax.experimental.shard_map import shard_map
    from concourse import bass2jax

    nc = _get_nc()
    bass2jax.install_neuronx_cc_hook()
    partition_name = (nc.partition_id_tensor.name
                      if nc.partition_id_tensor else None)
    in_names, out_names, out_avals, zero_shapes = [], [], [], []
    for alloc in nc.m.functions[0].allocations:
        if not isinstance(alloc, mybir.MemoryLocationSet):
            continue
        name = alloc.memorylocations[0].name
        if alloc.kind == "ExternalInput":
            if name != partition_name:
                in_names.append(name)
        elif alloc.kind == "ExternalOutput":
            shape = tuple(alloc.tensor_shape)
            dtype = mybir.dt.np(alloc.dtype)
            out_names.append(name)
            out_avals.append(jax.core.ShapedArray(shape, dtype))
            zero_shapes.append((shape, dtype))
    n_params = len(in_names)
    bind_in_names = list(in_names) + list(out_names)
    if partition_name is not None:
        bind_in_names.append(partition_name)

    def _body(*args):
        operands = list(args)
        if partition_name is not None:
            operands.append(bass2jax.partition_id_tensor())
        outs = bass2jax._bass_exec_p.bind(
            *operands,
            out_avals=tuple(out_avals),
            in_names=tuple(bind_in_names),
            out_names=tuple(out_names),
            lowering_input_output_aliases=(),
            sim_require_finite=True,
            sim_require_nnan=True,
            nc=nc,
        )
        return tuple(outs)

    devices = jax.devices()[:NCORES]
    mesh = Mesh(np.asarray(devices), ("core",))
    in_specs = (PartitionSpec("core"),) * (n_params + len(out_names))
    out_specs = (PartitionSpec("core"),) * len(out_names)
    donate = tuple(range(n_params, n_params + len(out_names)))
    sharded = jax.jit(
        shard_map(_body, mesh=mesh, in_specs=in_specs, out_specs=out_specs,
                  check_rep=False),
        donate_argnums=donate, keep_unused=True)

    def run(in_maps):
        per_core = [[np.asarray(m[name]) for name in in_names]
                    for m in in_maps]
        concat_in = [np.concatenate([per_core[c][i] for c in range(NCORES)],
                                    axis=0) for i in range(n_params)]
        concat_zeros = [np.zeros((NCORES * s[0], *s[1:]), d)
                        for s, d in zero_shapes]
        out_arrs = sharded(*concat_in, *concat_zeros)
        return [
            {name: np.asarray(out_arrs[i]).reshape(
                NCORES, *out_avals[i].shape)[c]
             for i, name in enumerate(out_names)}
            for c in range(NCORES)]

    _CACHED_RUNNER = run
    return run


_TRI = None


def make_in_maps(x, W_attn, b_attn, W_proj):
    global _TRI
    x = np.asarray(x, np.float32)
    W_attn = np.asarray(W_attn, np.float32)
    b_attn = np.asarray(b_attn, np.float32)
    W_proj = np.asarray(W_proj, np.float32)
    scale = np.float32(1.0 / np.sqrt(D))
    if _TRI is None:
        _TRI = np.where(np.arange(128)[None, :] >= np.arange(128)[:, None],
                        np.float32(0.0), np.float32(-1e4)).astype(np.float32)
    xts = [x[b].T.astype(BF) for b in range(B)]
    in_maps = []
    for core in range(NCORES):
        b = core // 2
        hs = (core % 2) * FL
        qc = slice(hs, hs + FL)
        kc = slice(C + hs, C + hs + FL)
        vc = slice(2 * C + hs, 2 * C + hs + FL)
        wa = np.empty((C, 3 * FL), dtype=BF)
        np.multiply(W_attn[:, qc], scale, out=wa[:, 0:FL], casting="unsafe")
        wa[:, FL:2 * FL] = W_attn[:, kc]
        wa[:, 2 * FL:3 * FL] = W_attn[:, vc]
        in_maps.append({
            "xt": xts[b],
            "wa": wa,
            "wp": W_proj[hs:hs + FL, :].astype(BF),
            "bq": np.ascontiguousarray((b_attn[qc] * scale)
                                       .reshape(4, 128).T.astype(np.float32)),
            "tri": _TRI,
        })
    return in_maps


def kernel(x, W_attn, b_attn, W_proj, b_proj):
    x = np.asarray(x, np.float32)
    W_attn = np.asarray(W_attn, np.float32)
    b_attn = np.asarray(b_attn, np.float32)
    W_proj = np.asarray(W_proj, np.float32)
    b_proj = np.asarray(b_proj, np.float32)

    run = _get_runner()
    in_maps = make_in_maps(x, W_attn, b_attn, W_proj)
    res = run(in_maps)
    outs = [res[i]["out"].astype(np.float32) for i in range(NCORES)]
    y = np.stack([outs[2 * b] + outs[2 * b + 1] for b in range(B)])
    # v-bias folds through attention (rows sum to 1) into a constant output
    # bias: b_proj + b_v @ W_proj.  k-bias is softmax-invariant (dropped).
    bias_out = b_proj + b_attn[2 * C:] @ W_proj
    return (y + bias_out[None, None, :]).astype(np.float32)
